# revision 39
# baseline (speedup 1.0000x reference)
"""AffinityLoss (torchdistill) Trainium2 kernel.

loss = mean_b [ sum_c sqrt(2 - 2*cos^2(s_bc, t_bc)) / HW ]

with s_bc, t_bc the HW-dim spatial vectors of channel c of sample b.
cos(s, t) = <s,t> / (||s|| ||t||), so per (b, c) we only need the three
dot products ss, tt, st over the 16384-element spatial dim.

Sharding: data-parallel over the batch dim B=8 -> one sample per
NeuronCore (8 cores). Per core, channels live on SBUF partitions
(2 chunks of 128) and the spatial dim is tiled along the free dim.

The kernel is a pure streaming 3-accumulator reduction, engine-split so
every engine stays well under the ~430 GB/s DMA feed rate:
  - ScalarE (ACT):  Square(s) accum -> ss; t^2 for most cols   (~57%)
  - VectorE (DVE):  stt(s*t) accum -> st; t^2 for cols 1,3,5   (~54%)
Both write their full-size `out` through a stride-0 broadcast dummy
(no scratch SBUF, no write bandwidth). Per spatial tile each engine
drops one fp32 partial per channel into its column of a [128, 3*NCOL]
accumulator tile; two tiny DMAs ship it to HBM (the cc0 block mid-
stream, the cc1 block at the end) and the host finishes the per-channel
closed form sqrt(2 - 2*st^2/(ss*tt)) in float64 (2048 channels/core).

The wide phase streams 4 MiB tiles from a 2-deep F=8192 pool (few ops,
low fixed overhead); the tapered tail (2048 ... 512) lives in its own
4-deep F=2048 pool, so the final DMAs are never gated on the wide
phase's st-chain through buffer reuse, and the last column's t^2 stays
on ACT so only ~1us of compute remains after the last byte lands.
"""

import numpy as np

import concourse.bacc as bacc
import concourse.tile as tile
from concourse import mybir
from concourse.bass_utils import run_bass_kernel_spmd

B, C, H, W = 8, 256, 128, 128
HW = H * W           # 16384 spatial elements per channel
P = 128              # SBUF partitions
NCORES = 8

FW = 8192            # wide tile width  (per-DMA: 128 x 8192 x 4B = 4 MiB)
FT = 2048            # tail tile width  (per-DMA: 1 MiB)

# Per-channel-chunk spatial tile widths; the last cc tapers so the
# compute tail after the final DMA is short. Wide cols use a 2-deep
# F=8192 pool; tail cols a separate 4-deep F=2048 pool, so the tail
# DMAs never wait on the wide-phase st-chain through buffer reuse.
# The geometric taper keeps each engine's post-last-byte work ~0.6us:
# a col's 3 ops (~3.3*w ns over 2 engines) finish faster than its data
# arrives (~2.4*w ns/col-pair), so no backlog builds if the final cols
# are narrow.
WIDTHS_CC0 = [8192, 4096, 4096]
WIDTHS_CC1 = [4096, 4096, 2048, 2048, 2048, 1536, 512]

# Global col numbering: 0 = cc0 8192; 1,2 = cc0 4096 halves; 3,4 = cc1
# 4096 halves; 5..9 = [2048, 2048, 2048, 1536, 512].  Each pair of
# consecutive 4096 halves shares one io_w tile pair (SBUF-neutral).
# Splitting a wide col starts its t-dependent work (st+tt, ~16.6
# engine-us for a full 8192) ~10us earlier: cc0's col-1 burst landed
# at t=47us and cascaded DVE ~4us behind data through the whole cc1
# tail; with both wide cols (except the hoisted col 0) split, each
# engine stays data-gated into the endgame.
# Cols 0..6 emit per-col in program order; t^2 of cols 2, 4, 6 runs on
# DVE, the rest on ACT.  The 3 tail cols (7..9) get an explicit
# per-engine emission order (engines execute their queue in program
# order, so emission order IS the schedule):
#   ACT: ss7, tt7, ss8, tt8, tt9
#   DVE: st7, st8, ss9, st9
T2_ON_DVE = {2, 4, 6}
# (engine, kind, col): kind 0=ss, 1=tt, 2=st -- emission order for the
# tail cols; any interleaving that preserves each engine's subsequence.
TAIL_SCHED = [
    ("scalar", 0, 7), ("vector", 2, 7), ("scalar", 1, 7),
    ("scalar", 0, 8), ("vector", 2, 8), ("scalar", 1, 8),
    ("vector", 0, 9), ("vector", 2, 9), ("scalar", 1, 9),
]
TAIL_COLS = {c for _, _, c in TAIL_SCHED}


def _tile_schedule():
    sched = []
    col = 0
    col_ranges = []
    for cc, widths in enumerate((WIDTHS_CC0, WIDTHS_CC1)):
        assert sum(widths) == HW
        c0 = col
        off = 0
        for w in widths:
            sched.append((cc, off, w, col))
            off += w
            col += 1
        col_ranges.append((c0, col))
    return sched, col_ranges


_SCHED, _COL_RANGES = _tile_schedule()
NCOL = len(_SCHED)
NC0 = len(WIDTHS_CC0)
NC1 = len(WIDTHS_CC1)


def _acc_col(kind, col):
    """Column in the acc tile: [cc0: ss|tt|st, cc1-main: ss|tt|st,
    last col: ss,tt,st].  The final col's triple sits contiguously at
    the end so the last output DMA ships only those 12 B/partition."""
    if col < NC0:
        return kind * NC0 + col
    if col == NCOL - 1:
        return 3 * (NCOL - 1) + kind
    return 3 * NC0 + kind * (NC1 - 1) + (col - NC0)


def build_program(ncores=NCORES):
    f32 = mybir.dt.float32

    nc = bacc.Bacc("TRN2", target_bir_lowering=False, debug=False,
                   num_devices=ncores)
    s_d = nc.dram_tensor("student", [C, HW], f32, kind="ExternalInput").ap()
    t_d = nc.dram_tensor("teacher", [C, HW], f32, kind="ExternalInput").ap()
    out_d = nc.dram_tensor("out", [P, 3 * NCOL], f32, kind="ExternalOutput").ap()

    cc0_cols = 3 * NC0          # acc columns belonging to cc0

    with tile.TileContext(nc) as tc:
        with (
            tc.tile_pool(name="io_w", bufs=2) as io_w,
            tc.tile_pool(name="io_t", bufs=4) as io_t,
            tc.tile_pool(name="small", bufs=1) as small,
        ):
            acc = small.tile([P, 3 * NCOL], f32)
            dummy_act = small.tile([P, 1], f32)
            dummy_dve = small.tile([P, 1], f32)

            # Early touch: ACT Square rides in an activation table set --
            # touching it first makes the ~1.3us table load overlap the DMA
            # ramp instead of serializing after the first tile lands.
            nc.vector.memset(dummy_act, 1.0)
            nc.vector.memset(dummy_dve, 1.0)
            nc.scalar.activation(
                out=dummy_act, in_=dummy_act,
                func=mybir.ActivationFunctionType.Square,
            )

            def square(engine, dummy, src, accum):
                if engine == "scalar":
                    nc.scalar.activation(
                        out=dummy.broadcast_to(src.shape), in_=src,
                        func=mybir.ActivationFunctionType.Square,
                        accum_out=accum,
                    )
                else:
                    nc.vector.scalar_tensor_tensor(
                        out=dummy.broadcast_to(src.shape),
                        in0=src, scalar=1.0, in1=src,
                        op0=mybir.AluOpType.mult, op1=mybir.AluOpType.mult,
                        accum_out=accum,
                    )

            def stt(src0, src1, accum):
                # NOTE: tensor_tensor_reduce wedges the exec unit on this
                # runtime build; scalar_tensor_tensor + accum_out is the
                # same single-pass fused multiply-reduce on the DVE.
                nc.vector.scalar_tensor_tensor(
                    out=dummy_dve.broadcast_to(src0.shape),
                    in0=src0, scalar=1.0, in1=src1,
                    op0=mybir.AluOpType.mult, op1=mybir.AluOpType.mult,
                    accum_out=accum,
                )

            def acc_cols(col):
                return tuple(
                    acc[:, _acc_col(k, col):_acc_col(k, col) + 1]
                    for k in range(3)
                )

            tiles = {}
            n4096 = 0
            cur_s = cur_t = None
            for cc, off, w, col in _SCHED:
                if w == 4096:
                    # consecutive 4096 halves share one io_w tile pair;
                    # the AP-level dependency tracking keeps the halves'
                    # DMAs and computes independent.
                    if n4096 % 2 == 0:
                        g = n4096 // 2
                        cur_s = io_w.tile([P, FW], f32, tag="s",
                                          name=f"s_shared{g}")
                        cur_t = io_w.tile([P, FW], f32, tag="t",
                                          name=f"t_shared{g}")
                        half = slice(0, 4096)
                    else:
                        half = slice(4096, 8192)
                    n4096 += 1
                    s_ap = cur_s[:, half]
                    t_ap = cur_t[:, half]
                else:
                    pool, fw = (io_w, FW) if w > FT else (io_t, FT)
                    s_tile = pool.tile([P, fw], f32, tag="s")
                    t_tile = pool.tile([P, fw], f32, tag="t")
                    s_ap = s_tile[:, :w]
                    t_ap = t_tile[:, :w]
                nc.sync.dma_start(
                    out=s_ap,
                    in_=s_d[cc * P:(cc + 1) * P, off:off + w],
                )
                nc.sync.dma_start(
                    out=t_ap,
                    in_=t_d[cc * P:(cc + 1) * P, off:off + w],
                )
                tiles[col] = (s_ap, t_ap)

                if col in TAIL_COLS:
                    # tail cols: defer compute; the tail pool is deep
                    # enough that none of these buffers are reused.
                    continue

                ss_col, tt_col, st_col = acc_cols(col)
                square("scalar", dummy_act, tiles[col][0], ss_col)
                if col in T2_ON_DVE:
                    square("vector", dummy_dve, tiles[col][1], tt_col)
                else:
                    square("scalar", dummy_act, tiles[col][1], tt_col)
                stt(tiles[col][0], tiles[col][1], st_col)

            for eng, kind, col in TAIL_SCHED:
                s_ap, t_ap = tiles[col]
                tgt = acc_cols(col)[kind]
                if kind == 0:
                    square(eng, dummy_act if eng == "scalar" else dummy_dve,
                           s_ap, tgt)
                elif kind == 1:
                    square(eng, dummy_act if eng == "scalar" else dummy_dve,
                           t_ap, tgt)
                else:
                    stt(s_ap, t_ap, tgt)

            # The accumulators ship in two pieces, both emitted after
            # every input DMA so no output wait can ever head-of-line
            # block the SP FIFO mid-stream (a waiting out-DMA ahead of
            # tail input issues starves the queue for ~3.5us): the bulk
            # (cc0 + cc1 cols 2..7, gated on their accumulator drains)
            # on sync, and the final col's 12 B/partition triple on the
            # scalar HWDGE queue -- the only DMA that waits for the
            # very last reduction.
            main_cols = 3 * (NCOL - 1)
            nc.sync.dma_start(out=out_d[:, :main_cols],
                              in_=acc[:, :main_cols])
            nc.scalar.dma_start(out=out_d[:, main_cols:],
                                in_=acc[:, main_cols:])

    _hoist_first_dmas(nc)
    nc.finalize()
    return nc


def _hoist_first_dmas(nc):
    """Move the first two SP-issued input DMAs (col 0's s/t tiles, no
    waits) into the Bass preamble region, before the const-ap memsets
    and the all-engine barrier.  The SP sequencer then issues them at
    ~5.7us instead of ~7.2us (after tile-entry), so the input stream
    starts ~1.4us earlier.  Consumers still gate on the DMAs'
    completion semaphores, so ordering stays correct; the DMAs
    themselves have no waits and write freshly-allocated SBUF."""
    try:
        f = nc.main_func
        moved = []
        for b in f.blocks:
            for inst in b.instructions:
                if (type(inst).__name__ == "InstDMACopy"
                        and getattr(inst.engine, "name", None) == "SP"
                        and not (inst.sync_info and inst.sync_info.on_wait)):
                    moved.append((b, inst))
                    if len(moved) == 2:
                        break
            if len(moved) == 2:
                break
        if len(moved) != 2:
            return
        b0 = f.blocks[0]
        pos = next(i for i, inst in enumerate(b0.instructions)
                   if type(inst).__name__ == "InstMemset")
        for b, inst in reversed(moved):
            b.instructions.remove(inst)
            b0.instructions.insert(pos, inst)
    except Exception:
        pass  # fall back to the unhoisted (still-correct) program


_PROGRAM = None


def _get_program():
    global _PROGRAM
    if _PROGRAM is None:
        _PROGRAM = build_program()
    return _PROGRAM


def _host_epilogue(acc_list) -> float:
    """acc_list: per-core [128, 3*NCOL] fp32 accumulator columns in the
    cc-major layout. Finish in float64 and return the scalar loss."""
    total = 0.0
    for a in acc_list:
        a = np.asarray(a, dtype=np.float64)
        for cc, (c0, c1) in enumerate(_COL_RANGES):
            ss, tt, st = (
                a[:, [_acc_col(k, c) for c in range(c0, c1)]].sum(axis=1)
                for k in range(3)
            )
            cos2 = (st * st) / (ss * tt)
            w = np.sqrt(np.clip(2.0 - 2.0 * cos2, 0.0, None))
            total += float(w.sum())
    return total / (HW * B)


def kernel(student: np.ndarray, teacher: np.ndarray) -> np.ndarray:
    s = np.ascontiguousarray(np.asarray(student, dtype=np.float32)).reshape(B, C, HW)
    t = np.ascontiguousarray(np.asarray(teacher, dtype=np.float32)).reshape(B, C, HW)

    nc = _get_program()
    in_maps = [{"student": s[i], "teacher": t[i]} for i in range(NCORES)]
    results = run_bass_kernel_spmd(nc, in_maps, list(range(NCORES))).results

    total = _host_epilogue([results[i]["out"] for i in range(NCORES)])
    return np.asarray(total, dtype=np.float32)



# revision 40
# speedup vs baseline: 1.0598x; 1.0598x over previous
"""AffinityLoss (torchdistill) Trainium2 kernel.

loss = mean_b [ sum_c sqrt(2 - 2*cos^2(s_bc, t_bc)) / HW ]

with s_bc, t_bc the HW-dim spatial vectors of channel c of sample b.
cos(s, t) = <s,t> / (||s|| ||t||), so per (b, c) we only need the three
dot products ss, tt, st over the 16384-element spatial dim.

Sharding: data-parallel over the batch dim B=8 -> one sample per
NeuronCore (8 cores). Per core, channels live on SBUF partitions
(2 chunks of 128) and the spatial dim is tiled along the free dim.

The kernel is a pure streaming 3-accumulator reduction, engine-split so
every engine stays well under the ~430 GB/s DMA feed rate:
  - ScalarE (ACT):  Square(s) accum -> ss; t^2 for most cols   (~57%)
  - VectorE (DVE):  stt(s*t) accum -> st; t^2 for cols 1,3,5   (~54%)
Both write their full-size `out` through a stride-0 broadcast dummy
(no scratch SBUF, no write bandwidth). Per spatial tile each engine
drops one fp32 partial per channel into its column of a [128, 3*NCOL]
accumulator tile; two tiny DMAs ship it to HBM (the cc0 block mid-
stream, the cc1 block at the end) and the host finishes the per-channel
closed form sqrt(2 - 2*st^2/(ss*tt)) in float64 (2048 channels/core).

The wide phase streams 4 MiB tiles from a 2-deep F=8192 pool (few ops,
low fixed overhead); the tapered tail (2048 ... 512) lives in its own
4-deep F=2048 pool, so the final DMAs are never gated on the wide
phase's st-chain through buffer reuse, and the last column's t^2 stays
on ACT so only ~1us of compute remains after the last byte lands.
"""

import numpy as np

import concourse.bacc as bacc
import concourse.tile as tile
from concourse import mybir
from concourse.bass_utils import run_bass_kernel_spmd

B, C, H, W = 8, 256, 128, 128
HW = H * W           # 16384 spatial elements per channel
P = 128              # SBUF partitions
NCORES = 8

FW = 8192            # wide tile width  (per-DMA: 128 x 8192 x 4B = 4 MiB)
FT = 2048            # tail tile width  (per-DMA: 1 MiB)

# Per-channel-chunk spatial tile widths; the last cc tapers so the
# compute tail after the final DMA is short. Wide cols use a 2-deep
# F=8192 pool; tail cols a separate 4-deep F=2048 pool, so the tail
# DMAs never wait on the wide-phase st-chain through buffer reuse.
# The geometric taper keeps each engine's post-last-byte work ~0.6us:
# a col's 3 ops (~3.3*w ns over 2 engines) finish faster than its data
# arrives (~2.4*w ns/col-pair), so no backlog builds if the final cols
# are narrow.
WIDTHS_CC0 = [8192, 8192]
WIDTHS_CC1 = [4096, 4096, 2048, 2048, 2048, 1536, 512]

# Global col numbering: 0,1 = cc0 8192s; 2..8 = cc1 [4096, 4096, 2048,
# 2048, 2048, 1536, 512].  The cc1 wide col is split into two 4096
# halves sharing one io_w tile pair (SBUF-neutral): its t-dependent
# work (st+tt, ~16.6 engine-us for a full 8192) previously all landed
# at once ~20us before the stream end, creating the backlog that
# pushed both engines' end chains to T+3.5; the first half now lands
# ~10us earlier and interleaves with the stream.
# Cols 0..5 emit per-col in program order; t^2 of cols 1, 3 runs on
# DVE, the rest on ACT.  The 3 tail cols (6..8) get an explicit
# per-engine emission order (engines execute their queue in program
# order, so emission order IS the schedule):
#   ACT: ss6, tt6, ss7, tt7, tt8
#   DVE: st6, st7, ss8, st8
T2_ON_DVE = {1, 3}
# (engine, kind, col): kind 0=ss, 1=tt, 2=st -- emission order for the
# tail cols; any interleaving that preserves each engine's subsequence.
TAIL_SCHED = [
    ("scalar", 0, 6), ("vector", 2, 6), ("scalar", 1, 6),
    ("scalar", 0, 7), ("vector", 2, 7), ("scalar", 1, 7),
    ("vector", 0, 8), ("vector", 2, 8), ("scalar", 1, 8),
]
TAIL_COLS = {c for _, _, c in TAIL_SCHED}


def _tile_schedule():
    sched = []
    col = 0
    col_ranges = []
    for cc, widths in enumerate((WIDTHS_CC0, WIDTHS_CC1)):
        assert sum(widths) == HW
        c0 = col
        off = 0
        for w in widths:
            sched.append((cc, off, w, col))
            off += w
            col += 1
        col_ranges.append((c0, col))
    return sched, col_ranges


_SCHED, _COL_RANGES = _tile_schedule()
NCOL = len(_SCHED)
NC0 = len(WIDTHS_CC0)
NC1 = len(WIDTHS_CC1)


def _acc_col(kind, col):
    """Column in the acc tile: [cc0: ss|tt|st, cc1-main: ss|tt|st,
    last col: ss,tt,st].  The final col's triple sits contiguously at
    the end so the last output DMA ships only those 12 B/partition."""
    if col < NC0:
        return kind * NC0 + col
    if col == NCOL - 1:
        return 3 * (NCOL - 1) + kind
    return 3 * NC0 + kind * (NC1 - 1) + (col - NC0)


def build_program(ncores=NCORES):
    f32 = mybir.dt.float32

    nc = bacc.Bacc("TRN2", target_bir_lowering=False, debug=False,
                   num_devices=ncores)
    s_d = nc.dram_tensor("student", [C, HW], f32, kind="ExternalInput").ap()
    t_d = nc.dram_tensor("teacher", [C, HW], f32, kind="ExternalInput").ap()
    out_d = nc.dram_tensor("out", [P, 3 * NCOL], f32, kind="ExternalOutput").ap()

    cc0_cols = 3 * NC0          # acc columns belonging to cc0

    with tile.TileContext(nc) as tc:
        with (
            tc.tile_pool(name="io_w", bufs=2) as io_w,
            tc.tile_pool(name="io_t", bufs=4) as io_t,
            tc.tile_pool(name="small", bufs=1) as small,
        ):
            acc = small.tile([P, 3 * NCOL], f32)
            dummy_act = small.tile([P, 1], f32)
            dummy_dve = small.tile([P, 1], f32)

            # Early touch: ACT Square rides in an activation table set --
            # touching it first makes the ~1.3us table load overlap the DMA
            # ramp instead of serializing after the first tile lands.
            nc.vector.memset(dummy_act, 1.0)
            nc.vector.memset(dummy_dve, 1.0)
            nc.scalar.activation(
                out=dummy_act, in_=dummy_act,
                func=mybir.ActivationFunctionType.Square,
            )

            def square(engine, dummy, src, accum):
                if engine == "scalar":
                    nc.scalar.activation(
                        out=dummy.broadcast_to(src.shape), in_=src,
                        func=mybir.ActivationFunctionType.Square,
                        accum_out=accum,
                    )
                else:
                    nc.vector.scalar_tensor_tensor(
                        out=dummy.broadcast_to(src.shape),
                        in0=src, scalar=1.0, in1=src,
                        op0=mybir.AluOpType.mult, op1=mybir.AluOpType.mult,
                        accum_out=accum,
                    )

            def stt(src0, src1, accum):
                # NOTE: tensor_tensor_reduce wedges the exec unit on this
                # runtime build; scalar_tensor_tensor + accum_out is the
                # same single-pass fused multiply-reduce on the DVE.
                nc.vector.scalar_tensor_tensor(
                    out=dummy_dve.broadcast_to(src0.shape),
                    in0=src0, scalar=1.0, in1=src1,
                    op0=mybir.AluOpType.mult, op1=mybir.AluOpType.mult,
                    accum_out=accum,
                )

            def acc_cols(col):
                return tuple(
                    acc[:, _acc_col(k, col):_acc_col(k, col) + 1]
                    for k in range(3)
                )

            tiles = {}
            shared = {}
            for cc, off, w, col in _SCHED:
                if w == 4096:
                    # the two 4096 halves share one io_w tile pair; the
                    # AP-level dependency tracking keeps the halves'
                    # DMAs and computes independent.
                    if "s" not in shared:
                        shared["s"] = io_w.tile([P, FW], f32, tag="s",
                                                name="s_shared")
                        shared["t"] = io_w.tile([P, FW], f32, tag="t",
                                                name="t_shared")
                        half = slice(0, 4096)
                    else:
                        half = slice(4096, 8192)
                    s_ap = shared["s"][:, half]
                    t_ap = shared["t"][:, half]
                else:
                    pool, fw = (io_w, FW) if w > FT else (io_t, FT)
                    s_tile = pool.tile([P, fw], f32, tag="s")
                    t_tile = pool.tile([P, fw], f32, tag="t")
                    s_ap = s_tile[:, :w]
                    t_ap = t_tile[:, :w]
                nc.sync.dma_start(
                    out=s_ap,
                    in_=s_d[cc * P:(cc + 1) * P, off:off + w],
                )
                nc.sync.dma_start(
                    out=t_ap,
                    in_=t_d[cc * P:(cc + 1) * P, off:off + w],
                )
                tiles[col] = (s_ap, t_ap)

                if col in TAIL_COLS:
                    # tail cols: defer compute; the tail pool is deep
                    # enough that none of these buffers are reused.
                    continue

                ss_col, tt_col, st_col = acc_cols(col)
                square("scalar", dummy_act, tiles[col][0], ss_col)
                if col in T2_ON_DVE:
                    square("vector", dummy_dve, tiles[col][1], tt_col)
                else:
                    square("scalar", dummy_act, tiles[col][1], tt_col)
                stt(tiles[col][0], tiles[col][1], st_col)

            for eng, kind, col in TAIL_SCHED:
                s_ap, t_ap = tiles[col]
                tgt = acc_cols(col)[kind]
                if kind == 0:
                    square(eng, dummy_act if eng == "scalar" else dummy_dve,
                           s_ap, tgt)
                elif kind == 1:
                    square(eng, dummy_act if eng == "scalar" else dummy_dve,
                           t_ap, tgt)
                else:
                    stt(s_ap, t_ap, tgt)

            # The accumulators ship in two pieces, both emitted after
            # every input DMA so no output wait can ever head-of-line
            # block the SP FIFO mid-stream (a waiting out-DMA ahead of
            # tail input issues starves the queue for ~3.5us): the bulk
            # (cc0 + cc1 cols 2..7, gated on their accumulator drains)
            # on sync, and the final col's 12 B/partition triple on the
            # scalar HWDGE queue -- the only DMA that waits for the
            # very last reduction.
            main_cols = 3 * (NCOL - 1)
            nc.sync.dma_start(out=out_d[:, :main_cols],
                              in_=acc[:, :main_cols])
            nc.scalar.dma_start(out=out_d[:, main_cols:],
                                in_=acc[:, main_cols:])

    _hoist_first_dmas(nc)
    nc.finalize()
    return nc


def _hoist_first_dmas(nc):
    """Move the first two SP-issued input DMAs (col 0's s/t tiles, no
    waits) into the Bass preamble region, before the const-ap memsets
    and the all-engine barrier.  The SP sequencer then issues them at
    ~5.7us instead of ~7.2us (after tile-entry), so the input stream
    starts ~1.4us earlier.  Consumers still gate on the DMAs'
    completion semaphores, so ordering stays correct; the DMAs
    themselves have no waits and write freshly-allocated SBUF."""
    try:
        f = nc.main_func
        moved = []
        for b in f.blocks:
            for inst in b.instructions:
                if (type(inst).__name__ == "InstDMACopy"
                        and getattr(inst.engine, "name", None) == "SP"
                        and not (inst.sync_info and inst.sync_info.on_wait)):
                    moved.append((b, inst))
                    if len(moved) == 2:
                        break
            if len(moved) == 2:
                break
        if len(moved) != 2:
            return
        b0 = f.blocks[0]
        pos = next(i for i, inst in enumerate(b0.instructions)
                   if type(inst).__name__ == "InstMemset")
        for b, inst in reversed(moved):
            b.instructions.remove(inst)
            b0.instructions.insert(pos, inst)
    except Exception:
        pass  # fall back to the unhoisted (still-correct) program


_PROGRAM = None


def _get_program():
    global _PROGRAM
    if _PROGRAM is None:
        _PROGRAM = build_program()
    return _PROGRAM


def _host_epilogue(acc_list) -> float:
    """acc_list: per-core [128, 3*NCOL] fp32 accumulator columns in the
    cc-major layout. Finish in float64 and return the scalar loss."""
    total = 0.0
    for a in acc_list:
        a = np.asarray(a, dtype=np.float64)
        for cc, (c0, c1) in enumerate(_COL_RANGES):
            ss, tt, st = (
                a[:, [_acc_col(k, c) for c in range(c0, c1)]].sum(axis=1)
                for k in range(3)
            )
            cos2 = (st * st) / (ss * tt)
            w = np.sqrt(np.clip(2.0 - 2.0 * cos2, 0.0, None))
            total += float(w.sum())
    return total / (HW * B)


def kernel(student: np.ndarray, teacher: np.ndarray) -> np.ndarray:
    s = np.ascontiguousarray(np.asarray(student, dtype=np.float32)).reshape(B, C, HW)
    t = np.ascontiguousarray(np.asarray(teacher, dtype=np.float32)).reshape(B, C, HW)

    nc = _get_program()
    in_maps = [{"student": s[i], "teacher": t[i]} for i in range(NCORES)]
    results = run_bass_kernel_spmd(nc, in_maps, list(range(NCORES))).results

    total = _host_epilogue([results[i]["out"] for i in range(NCORES)])
    return np.asarray(total, dtype=np.float32)

